# revision 3
# baseline (speedup 1.0000x reference)
"""GCN autoencoder forward pass on 8 Trainium2 NeuronCores (Bass/Tile).

v2 strategy (graph/data parallel):
  - Nodes dealt to 8 cores in degree-sorted 128-row tiles (as v1), then
    within each core rows are regrouped into tiles by lexicographic
    source-chunk profile so per-(tile,chunk) wave counts stay tight.
  - Each conv layer: per-core matmul of its shard (table rows
    m = dinv * (act @ W), cast to fp16 for 128-wide tables / fp32 for
    the 64-wide one), AllGather into Shared-scratchpad tables, then
    per-core aggregation with batched InstDMAGatherAnt gathers: one
    instruction per (tile-block, table-chunk) moving thousands of rows
    (994ns SWDGE overhead amortized ~50x vs per-wave indirect DMA).
    Sentinel slots index a reserved all-zero table row per chunk.
  - Gathered segments are reduced per tile with pairwise adds
    alternating between the Vector and Pool ALUs; self-loop is folded
    into the gather as one more source.

Self-contained: includes the walrus sync-wait compat shim and a PJRT
runner (axon) replicating bass2jax.run_bass_via_pjrt.
"""

import sys

for _p in ("/opt/trn_rl_repo",):
    if _p not in sys.path:
        sys.path.insert(0, _p)

import numpy as np

import concourse.bass as bass
import concourse.mybir as mybir
import concourse.tile as tile
from concourse import library_config
from concourse.masks import make_identity

P = 128
NCORES = 8
N = 100000
TILES_PER_CORE = 98
SHARD = TILES_PER_CORE * P          # 12544
NPAD = NCORES * SHARD               # 100352
NTILES = NCORES * TILES_PER_CORE
F1, F2, FZ, FO = 128, 64, 5, 128
EPS = 1e-5
AF = mybir.ActivationFunctionType

NCHUNK = 4
CHUNK_WIN = NPAD // NCHUNK          # 25088 rows (2 shards) <= int16 range
RESERVED = [c * CHUNK_WIN for c in range(NCHUNK)]  # zero rows, local idx 0
SEGCAP = 160                        # max gathered segments per block
WCAP = 7                            # max waves per dma_gather: 57 ring entries, 2 fit per 128-entry queue ring


def make_blocks(waves):
    """Group tiles into blocks with ~SEGCAP total segments each.
    Deterministic from `waves` so plan and program agree."""
    wsum = np.asarray(waves).sum(axis=1)
    blocks, cur, tot = [], [], 0
    for i in range(TILES_PER_CORE):
        if cur and tot + int(wsum[i]) > SEGCAP:
            blocks.append(cur)
            cur, tot = [], 0
        cur.append(i)
        tot += int(wsum[i])
    if cur:
        blocks.append(cur)
    return blocks

# ---------------------------------------------------------------- compat ----

MAX_WAITS = 1


def _split_sync_waits(nc, max_waits=MAX_WAITS):
    """This container's walrus rejects >1 sync wait per instruction; move
    excess waits onto same-engine NOPs placed just before the offender."""
    for fn in nc.m.functions:
        for bb in fn.blocks:
            out = []
            for inst in bb.instructions:
                si = inst.sync_info
                if si is not None and si.on_wait and len(si.on_wait) > max_waits:
                    waits = list(si.on_wait)
                    head, tail = waits[:-max_waits], waits[-max_waits:]
                    for i in range(0, len(head), max_waits):
                        out.append(
                            mybir.InstNoOp(
                                name=f"{inst.name}-ws{i}",
                                engine=inst.engine,
                                bass_nofuse=True,
                                sync_info=mybir.SyncInfo(
                                    on_wait=head[i : i + max_waits], on_update=[]
                                ),
                            )
                        )
                    si.on_wait = tail
                out.append(inst)
            bb.instructions[:] = out


class CompatTileContext(tile.TileContext):
    def __exit__(self, *args):
        ret = super().__exit__(*args)
        _split_sync_waits(self.nc)
        return ret


# ---------------------------------------------------------------- runner ----


class SpmdRunner:
    def __init__(self, nc, n_cores=NCORES):
        import jax
        from jax.sharding import Mesh, PartitionSpec, NamedSharding
        from jax.experimental.shard_map import shard_map
        from concourse import bass2jax
        from concourse.bass2jax import _bass_exec_p, install_neuronx_cc_hook

        install_neuronx_cc_hook()
        mybir.codegen_inst_isa_subclasses(nc)
        self.jax = jax
        self.nc = nc
        self.n_cores = n_cores
        partition_name = (
            nc.partition_id_tensor.name if nc.partition_id_tensor else None
        )

        in_names, out_names, out_avals, zero_outs = [], [], [], []
        for alloc in nc.m.functions[0].allocations:
            if not isinstance(alloc, mybir.MemoryLocationSet):
                continue
            name = alloc.memorylocations[0].name
            if alloc.kind == "ExternalInput":
                if name != partition_name:
                    in_names.append(name)
            elif alloc.kind == "ExternalOutput":
                out_names.append(name)
                shape = tuple(alloc.tensor_shape)
                dtype = mybir.dt.np(alloc.dtype)
                out_avals.append(jax.core.ShapedArray(shape, dtype))
                zero_outs.append(np.zeros(shape, dtype))
        self.in_names = in_names
        self.out_names = out_names
        self.out_avals = out_avals
        self.zero_outs = zero_outs
        n_params = len(in_names)
        all_in_names = in_names + out_names
        if partition_name is not None:
            all_in_names = all_in_names + [partition_name]

        def _body(*args):
            operands = list(args)
            if partition_name is not None:
                operands.append(bass2jax.partition_id_tensor())
            outs = _bass_exec_p.bind(
                *operands,
                out_avals=tuple(out_avals),
                in_names=tuple(all_in_names),
                out_names=tuple(out_names),
                lowering_input_output_aliases=(),
                sim_require_finite=True,
                sim_require_nnan=True,
                nc=nc,
            )
            return tuple(outs)

        devices = jax.devices()[:n_cores]
        self.mesh = Mesh(np.asarray(devices), ("core",))
        in_specs = (PartitionSpec("core"),) * (n_params + len(out_names))
        out_specs = (PartitionSpec("core"),) * len(out_names)
        self.sharding = NamedSharding(self.mesh, PartitionSpec("core"))
        self.fn = jax.jit(
            shard_map(_body, mesh=self.mesh, in_specs=in_specs,
                      out_specs=out_specs, check_rep=False),
            keep_unused=True,
        )
        self._dev_args = None

    def stage(self, in_maps):
        n = self.n_cores
        concat = [
            np.concatenate([np.asarray(in_maps[c][name]) for c in range(n)], axis=0)
            for name in self.in_names
        ]
        concat += [
            np.zeros((n * z.shape[0], *z.shape[1:]), z.dtype) for z in self.zero_outs
        ]
        self._dev_args = [self.jax.device_put(a, self.sharding) for a in concat]
        for a in self._dev_args:
            a.block_until_ready()

    def run(self):
        outs = self.fn(*self._dev_args)
        self.jax.block_until_ready(outs)
        return outs

    def results(self, outs):
        res = []
        for c in range(self.n_cores):
            d = {}
            for i, name in enumerate(self.out_names):
                full = np.asarray(outs[i])
                d[name] = full.reshape(self.n_cores, *self.out_avals[i].shape)[c]
            res.append(d)
        return res


# ------------------------------------------------------------------ plan ----


def build_plan(edge_index):
    src0 = np.asarray(edge_index[0], dtype=np.int64)
    dst0 = np.asarray(edge_index[1], dtype=np.int64)

    deg = np.bincount(dst0, minlength=N).astype(np.int64) + 1  # + self loop

    order = np.argsort(-deg, kind="stable")
    reserved_mask = np.zeros(NPAD, dtype=bool)
    reserved_mask[RESERVED] = True

    # Phase 1: deal degree-sorted 128-row groups round-robin to cores.
    core_nodes = [[] for _ in range(NCORES)]
    n_data_tiles = (N + P - 1) // P
    for t in range(n_data_tiles):
        core_nodes[t % NCORES].extend(order[t * P : (t + 1) * P].tolist())
    cap = [SHARD - int(reserved_mask[c * SHARD : (c + 1) * SHARD].sum())
           for c in range(NCORES)]
    # rebalance: cores holding a reserved slot may overflow by a node
    for c in range(NCORES):
        while len(core_nodes[c]) > cap[c]:
            spill = core_nodes[c].pop()
            tgt = min(range(NCORES), key=lambda k: len(core_nodes[k]) - cap[k])
            core_nodes[tgt].append(spill)
    for c in range(NCORES):
        assert len(core_nodes[c]) <= cap[c]

    # Provisional slot per node (to know each node's chunk): core fill order
    # skipping reserved slots. Final within-core order decided in phase 3,
    # but the CHUNK of a node only depends on its core (chunks are
    # 4-shard-aligned), so it is already fixed here.
    chunk_of_core = np.repeat(np.arange(NCHUNK), NCORES // NCHUNK)
    node_core = np.full(N, -1, np.int64)
    for c in range(NCORES):
        node_core[core_nodes[c]] = c
    node_chunk = chunk_of_core[node_core]

    # Phase 2: per-destination per-chunk in-edge counts (self included).
    cnt = np.zeros((N, NCHUNK), np.int32)
    np.add.at(cnt, (dst0, node_chunk[src0]), 1)
    cnt[np.arange(N), node_chunk] += 1

    # Phase 3: within each core, order nodes by lexicographic chunk profile
    # (groups rows with similar per-chunk counts -> tight tile maxima),
    # then assign to slots skipping reserved ones.
    new_id = np.full(N, -1, np.int64)
    old_of_new = np.full(NPAD, -1, np.int64)
    for c in range(NCORES):
        nodes = np.asarray(core_nodes[c], np.int64)
        prof = cnt[nodes]
        o = np.lexsort(tuple(prof[:, k] for k in reversed(range(NCHUNK))))
        nodes = nodes[o]
        slots = np.arange(c * SHARD, (c + 1) * SHARD)
        slots = slots[~reserved_mask[slots]][: len(nodes)]
        new_id[nodes] = slots
        old_of_new[slots] = nodes

    nsrc = new_id[src0]
    ndst = new_id[dst0]

    # CSR by destination slot, sources sorted by chunk (ascending id works:
    # chunks are contiguous id ranges).
    order_e = np.argsort(ndst * (NPAD + 1) + nsrc, kind="stable")
    s_sorted = nsrc[order_e]
    d_sorted = ndst[order_e]
    ptr = np.zeros(NPAD + 1, dtype=np.int64)
    np.cumsum(np.bincount(ndst, minlength=NPAD), out=ptr[1:])

    # per-slot per-chunk counts (self included)
    scnt = np.zeros((NPAD, NCHUNK), np.int32)
    assigned = old_of_new >= 0
    scnt[assigned] = cnt[old_of_new[assigned]]

    # waves per (tile position, chunk): max over the tile's rows, then max
    # over cores (SPMD program shared by all cores).
    tile_rows = scnt.reshape(NCORES, TILES_PER_CORE, P, NCHUNK)
    waves = tile_rows.max(axis=2).max(axis=0).astype(np.int64)  # [TPC, NCHUNK]

    # flat index stream per core: for block b, for chunk k, for tile i in b,
    # for wave w, for partition p -> local chunk index (sentinel = 0).
    blocks = make_blocks(waves)
    core_flat = []
    for c in range(NCORES):
        flats = []
        for blk in blocks:
            for k in range(NCHUNK):
                base = k * CHUNK_WIN
                for i in blk:
                    W = int(waves[i, k])
                    if W == 0:
                        continue
                    sl = np.zeros((W, P), np.int64)  # sentinel local idx 0
                    for p in range(P):
                        g = c * SHARD + i * P + p
                        lo, hi = ptr[g], ptr[g + 1]
                        srcs = s_sorted[lo:hi]
                        # insert self (slot g) among this row's sources
                        srcs = np.sort(np.append(srcs, g))
                        sk = srcs[(srcs >= base) & (srcs < base + CHUNK_WIN)]
                        sl[: len(sk), p] = sk - base
                    flats.append(sl.ravel())
        flat = np.concatenate(flats) if flats else np.zeros(0, np.int64)
        core_flat.append(flat)

    flat_len = len(core_flat[0])
    assert all(len(f) == flat_len for f in core_flat)
    pad16 = (-flat_len) % 16
    cols16 = (flat_len + pad16) // 16

    core_idx = []
    for c in range(NCORES):
        f = np.concatenate([core_flat[c], np.zeros(pad16, np.int64)])
        wrapped = f.reshape(cols16, 16).T            # [16, cols16]
        wrapped = np.tile(wrapped, (8, 1))           # replicate to 128 parts
        core_idx.append(wrapped.astype(np.uint16).view(np.int16).copy())

    dinv = np.zeros(NPAD, dtype=np.float64)
    real = old_of_new >= 0
    dinv[real] = 1.0 / np.sqrt(deg[old_of_new[real]].astype(np.float64))
    dinv = dinv.astype(np.float32)
    dinv_cols = [
        dinv[c * SHARD : (c + 1) * SHARD].reshape(TILES_PER_CORE, P).T.copy()
        for c in range(NCORES)
    ]
    return {
        "old_of_new": old_of_new,
        "waves": waves,                    # [TPC, NCHUNK]
        "cols16": cols16,
        "core_idx": core_idx,
        "dinv_cols": dinv_cols,
    }


# ---------------------------------------------------------------- program ---


def build_program(waves, cols16, reps=1):
    """waves: [TILES_PER_CORE, NCHUNK] ndarray of wave counts."""
    nc = bass.Bass("TRN2", target_bir_lowering=False, debug=False,
                   enable_asserts=True, num_devices=NCORES,
                   num_swdge_queues=4)
    waves = np.asarray(waves, np.int64)
    wsum = waves.sum(axis=1)  # segments per tile

    x_s = nc.dram_tensor("x_shard", [SHARD, F1], mybir.dt.float32, kind="ExternalInput").ap()
    idx = nc.dram_tensor("idx", [P, cols16], mybir.dt.int16, kind="ExternalInput").ap()
    dinv_c = nc.dram_tensor("dinv_cols", [P, TILES_PER_CORE], mybir.dt.float32, kind="ExternalInput").ap()
    W1e = nc.dram_tensor("W1e", [F1, F1], mybir.dt.float32, kind="ExternalInput").ap()
    W2e = nc.dram_tensor("W2e", [F1, F2], mybir.dt.float32, kind="ExternalInput").ap()
    Wm = nc.dram_tensor("Wm", [F2, FZ], mybir.dt.float32, kind="ExternalInput").ap()
    W1d = nc.dram_tensor("W1d", [FZ, F1], mybir.dt.float32, kind="ExternalInput").ap()
    W2d = nc.dram_tensor("W2d", [F1, FO], mybir.dt.float32, kind="ExternalInput").ap()
    biases = nc.dram_tensor("biases", [P, F1 + F2 + FZ + F1 + FO + 2 * FZ],
                            mybir.dt.float32, kind="ExternalInput").ap()
    ident_in = nc.dram_tensor("ident", [P, P], mybir.dt.float32, kind="ExternalInput").ap()
    out_t = nc.dram_tensor("out", [SHARD, FO], mybir.dt.float32, kind="ExternalOutput").ap()

    bounce1 = nc.dram_tensor("bounce1", [SHARD, F1], mybir.dt.float16).ap()
    bounce2 = nc.dram_tensor("bounce2", [SHARD, F2], mybir.dt.float32).ap()
    bounce3 = nc.dram_tensor("bounce3", [SHARD, F1], mybir.dt.float16).ap()
    bounce4 = nc.dram_tensor("bounce4", [SHARD, FO], mybir.dt.float16).ap()
    t1 = nc.dram_tensor("t1", [NPAD, F1], mybir.dt.float16).ap()
    t2 = nc.dram_tensor("t2", [NPAD, F2], mybir.dt.float32).ap()
    t3 = nc.dram_tensor("t3", [NPAD, F1], mybir.dt.float16).ap()
    t4 = nc.dram_tensor("t4", [NPAD, FO], mybir.dt.float16).ap()

    rg = [list(range(NCORES))]
    OB1, OB2, OBM, OB1D, OB2D = 0, F1, F1 + F2, F1 + F2 + FZ, F1 + F2 + FZ + F1
    OLNW = OB2D + FO
    OLNB = OLNW + FZ

    blocks = make_blocks(waves)
    maxseg = max(int(sum(wsum[i] for i in blk)) for blk in blocks)

    # column offset (in idx unit-of-16) per (block, chunk, tile) in the
    # flat stream built by build_plan
    colpos = {}
    pos = 0
    for bi, blk in enumerate(blocks):
        for k in range(NCHUNK):
            for i in blk:
                colpos[(bi, k, i)] = pos
                pos += int(waves[i, k]) * P
    assert pos <= cols16 * 16

    nc.gpsimd.load_library(library_config.mlp)
    with CompatTileContext(nc) as tc:
        with (
            tc.tile_pool(name="const", bufs=1) as constp,
            tc.tile_pool(name="work", bufs=3) as workp,
            tc.tile_pool(name="gath", bufs=3) as gathp,
            tc.tile_pool(name="psum", bufs=2, space="PSUM") as psump,
        ):
            # identity uploaded from host: Pool runs the mlp Q7 library
            # (for dma_gather), which lacks the standard Pool ALU ops that
            # masks.make_identity needs.
            ident = constp.tile([P, P], mybir.dt.float32)
            nc.sync.dma_start(out=ident[:], in_=ident_in[:])
            idx_t = constp.tile([P, cols16], mybir.dt.int16)
            nc.sync.dma_start(out=idx_t[:], in_=idx[:])
            dinv_t = constp.tile([P, TILES_PER_CORE], mybir.dt.float32)
            nc.sync.dma_start(out=dinv_t[:], in_=dinv_c[:])
            w1e_t = constp.tile([F1, F1], mybir.dt.float32)
            nc.sync.dma_start(out=w1e_t[:], in_=W1e[:])
            w2e_t = constp.tile([F1, F2], mybir.dt.float32)
            nc.sync.dma_start(out=w2e_t[:], in_=W2e[:])
            wm_t = constp.tile([F2, FZ], mybir.dt.float32)
            nc.sync.dma_start(out=wm_t[:], in_=Wm[:])
            w1d_t = constp.tile([FZ, F1], mybir.dt.float32)
            nc.sync.dma_start(out=w1d_t[:], in_=W1d[:])
            w2d_t = constp.tile([F1, FO], mybir.dt.float32)
            nc.sync.dma_start(out=w2d_t[:], in_=W2d[:])
            bias_t = constp.tile([P, F1 + F2 + FZ + F1 + FO + 2 * FZ], mybir.dt.float32)
            nc.sync.dma_start(out=bias_t[:], in_=biases[:])
            inv5_t = constp.tile([P, 1], mybir.dt.float32)
            nc.vector.memset(inv5_t[:], 1.0 / FZ)
            eps_t = constp.tile([P, 1], mybir.dt.float32)
            nc.vector.memset(eps_t[:], EPS)

            def produce(i, act_tile, fin, w_t, fout, dst_bounce, tdt):
                tr = psump.tile([fin, P], mybir.dt.float32, tag="tr")
                nc.tensor.transpose(out=tr[:], in_=act_tile[:, :fin], identity=ident[:])
                trs = workp.tile([fin, P], mybir.dt.float32, tag="trs")
                nc.scalar.activation(trs[:], tr[:], AF.Copy)
                mm = psump.tile([P, fout], mybir.dt.float32, tag="mm")
                nc.tensor.matmul(mm[:], lhsT=trs[:], rhs=w_t[:, :fout], start=True, stop=True)
                ms = workp.tile([P, fout], tdt, tag="ms")
                nc.scalar.activation(ms[:], mm[:], AF.Copy, scale=dinv_t[:, i : i + 1])
                nc.sync.dma_start(out=dst_bounce[i * P : (i + 1) * P, :], in_=ms[:])

            GBYTES = maxseg * F1 * 2  # same byte size for fp16x128 / fp32x64
            qctr = [0]
            regcache = {}

            def nidx_reg(v):
                if v not in regcache:
                    regcache[v] = nc.gpsimd.to_reg(v)
                return regcache[v]

            def agg_blocks(table, fout, tdt, finish):
                """Aggregate all tiles: per (block, chunk) one dma_gather,
                then per-tile pairwise reductions alternating DVE/Pool."""
                esz = 2 if tdt == mybir.dt.float16 else 4
                for bi, blk in enumerate(blocks):
                    graw = gathp.tile([P, GBYTES], mybir.dt.int8, tag="g")
                    gf = graw[:].bitcast(tdt)  # [P, GBYTES//esz]

                    def seg2(a, b):  # flat 2D view of segments [a, b)
                        return gf[:, a * fout : b * fout]

                    # seg offset of (tile, chunk) within the block buffer:
                    # chunk-major then tile (matches flat stream)
                    segoff = {}
                    s = 0
                    for k in range(NCHUNK):
                        for i in blk:
                            segoff[(i, k)] = s
                            s += int(waves[i, k])
                    for k in range(NCHUNK):
                        Wk = int(sum(waves[i, k] for i in blk))
                        if Wk == 0:
                            continue
                        col0 = colpos[(bi, k, blk[0])]
                        s0 = segoff[(blk[0], k)]
                        # HW wedges on dma_gather above ~1024 indices
                        # (SWDGE ring entries = num_idxs/16+1): cap at 8
                        # waves and rotate queues to overlap gen/drain.
                        for w0 in range(0, Wk, WCAP):
                            wn = min(WCAP, Wk - w0)
                            nidx = wn * P
                            nreg = nidx_reg(nidx)
                            nc.gpsimd.dma_gather(
                                out_ap=seg2(s0 + w0, s0 + w0 + wn).rearrange(
                                    "p (s f) -> p s f", f=fout),
                                in_ap=table[k * CHUNK_WIN : (k + 1) * CHUNK_WIN],
                                idxs_ap=idx_t[:, (col0 + w0 * P) // 16 :
                                              (col0 + (w0 + wn) * P) // 16],
                                num_idxs=nidx,
                                num_idxs_reg=nreg,
                                elem_size=fout,
                                queue_num=qctr[0] % 4,
                            )
                            qctr[0] += 1
                    for i in blk:
                        eng = nc.vector  # Pool ALU unavailable under mlp lib
                        # reduce each chunk's segment range down to its head
                        # by pairwise halving, then combine chunk heads
                        heads = []
                        for k in range(NCHUNK):
                            W = int(waves[i, k])
                            if W == 0:
                                continue
                            s0 = segoff[(i, k)]
                            h = W
                            while h > 1:
                                m = h // 2
                                eng.tensor_tensor(
                                    out=seg2(s0, s0 + m),
                                    in0=seg2(s0, s0 + m),
                                    in1=seg2(s0 + h - m, s0 + h),
                                    op=mybir.AluOpType.add,
                                )
                                h -= m
                            heads.append(s0)
                        acc = workp.tile([P, fout], mybir.dt.float32, tag="acc")
                        if len(heads) == 1:
                            nc.scalar.activation(
                                acc[:], seg2(heads[0], heads[0] + 1), AF.Copy)
                        else:
                            eng.tensor_tensor(
                                out=acc[:], in0=seg2(heads[0], heads[0] + 1),
                                in1=seg2(heads[1], heads[1] + 1),
                                op=mybir.AluOpType.add)
                            for hd in heads[2:]:
                                eng.tensor_tensor(
                                    out=acc[:], in0=acc[:],
                                    in1=seg2(hd, hd + 1),
                                    op=mybir.AluOpType.add)
                        finish(i, acc)

            for _rep in range(reps):
                # ---- L1 produce
                for i in range(TILES_PER_CORE):
                    xa = workp.tile([P, F1], mybir.dt.float32, tag="xa")
                    nc.sync.dma_start(out=xa[:], in_=x_s[i * P : (i + 1) * P, :])
                    produce(i, xa, F1, w1e_t, F1, bounce1, mybir.dt.float16)
                nc.gpsimd.collective_compute(
                    "AllGather", mybir.AluOpType.bypass, replica_groups=rg,
                    ins=[bounce1[:]], outs=[t1[:]])

                # ---- L1 aggregate -> h (relu) -> L2 produce
                def fin1(i, acc):
                    e1 = workp.tile([P, F1], mybir.dt.float32, tag="epi1")
                    nc.scalar.activation(e1[:], acc[:], AF.Copy,
                                         scale=dinv_t[:, i : i + 1])
                    e2 = workp.tile([P, F1], mybir.dt.float32, tag="epi2")
                    nc.vector.tensor_tensor(
                        out=e2[:], in0=e1[:], in1=bias_t[:, OB1 : OB1 + F1],
                        op=mybir.AluOpType.add)
                    h = workp.tile([P, F1], mybir.dt.float32, tag="epi3")
                    nc.scalar.activation(h[:], e2[:], AF.Relu)
                    produce(i, h, F1, w2e_t, F2, bounce2, mybir.dt.float32)

                agg_blocks(t1, F1, mybir.dt.float16, fin1)
                nc.gpsimd.collective_compute(
                    "AllGather", mybir.AluOpType.bypass, replica_groups=rg,
                    ins=[bounce2[:]], outs=[t2[:]])

                # ---- L2 aggregate -> z -> bottleneck -> L3 produce
                def fin2(i, acc2):
                    zr = workp.tile([P, F2], mybir.dt.float32, tag="zrl")
                    # z = dinv*acc + b2e ; relu(z) fused: relu(dinv*acc + b)
                    zb = workp.tile([P, F2], mybir.dt.float32, tag="zb")
                    nc.scalar.activation(zb[:], acc2[:], AF.Copy,
                                         scale=dinv_t[:, i : i + 1])
                    nc.vector.tensor_tensor(
                        out=zr[:], in0=zb[:], in1=bias_t[:, OB2 : OB2 + F2],
                        op=mybir.AluOpType.add)
                    zrr = workp.tile([P, F2], mybir.dt.float32, tag="zrr")
                    nc.scalar.activation(zrr[:], zr[:], AF.Relu)
                    tr2 = psump.tile([F2, P], mybir.dt.float32, tag="tr")
                    nc.tensor.transpose(out=tr2[:], in_=zrr[:], identity=ident[:])
                    tr2s = workp.tile([F2, P], mybir.dt.float32, tag="trs")
                    nc.scalar.activation(tr2s[:], tr2[:], AF.Copy)
                    zm = psump.tile([P, FZ], mybir.dt.float32, tag="mm")
                    nc.tensor.matmul(zm[:], lhsT=tr2s[:], rhs=wm_t[:], start=True, stop=True)
                    zms = workp.tile([P, FZ], mybir.dt.float32, tag="zms")
                    nc.vector.tensor_tensor(
                        out=zms[:], in0=zm[:], in1=bias_t[:, OBM : OBM + FZ],
                        op=mybir.AluOpType.add)
                    musum = workp.tile([P, 1], mybir.dt.float32, tag="musum")
                    nc.vector.reduce_sum(musum[:], zms[:], axis=mybir.AxisListType.X)
                    mu = workp.tile([P, 1], mybir.dt.float32, tag="mu")
                    nc.vector.tensor_mul(out=mu[:], in0=musum[:], in1=inv5_t[:])
                    diff = workp.tile([P, FZ], mybir.dt.float32, tag="diff")
                    nc.vector.tensor_tensor(
                        out=diff[:], in0=zms[:], in1=mu[:].to_broadcast([P, FZ]),
                        op=mybir.AluOpType.subtract)
                    sq = workp.tile([P, FZ], mybir.dt.float32, tag="sq")
                    nc.vector.tensor_mul(out=sq[:], in0=diff[:], in1=diff[:])
                    varsum = workp.tile([P, 1], mybir.dt.float32, tag="varsum")
                    nc.vector.reduce_sum(varsum[:], sq[:], axis=mybir.AxisListType.X)
                    var = workp.tile([P, 1], mybir.dt.float32, tag="var")
                    nc.vector.tensor_mul(out=var[:], in0=varsum[:], in1=inv5_t[:])
                    vare = workp.tile([P, 1], mybir.dt.float32, tag="vare")
                    nc.vector.tensor_add(out=vare[:], in0=var[:], in1=eps_t[:])
                    sd = workp.tile([P, 1], mybir.dt.float32, tag="sd")
                    nc.scalar.activation(sd[:], vare[:], AF.Sqrt)
                    rinv = workp.tile([P, 1], mybir.dt.float32, tag="rinv")
                    nc.vector.reciprocal(rinv[:], sd[:])
                    zn = workp.tile([P, FZ], mybir.dt.float32, tag="zn")
                    nc.vector.tensor_mul(out=zn[:], in0=diff[:], in1=rinv[:].to_broadcast([P, FZ]))
                    zw = workp.tile([P, FZ], mybir.dt.float32, tag="zw")
                    nc.vector.tensor_mul(out=zw[:], in0=zn[:], in1=bias_t[:, OLNW : OLNW + FZ])
                    zl = workp.tile([P, FZ], mybir.dt.float32, tag="zl")
                    nc.vector.tensor_add(out=zl[:], in0=zw[:], in1=bias_t[:, OLNB : OLNB + FZ])
                    produce(i, zl, FZ, w1d_t, F1, bounce3, mybir.dt.float16)

                agg_blocks(t2, F2, mybir.dt.float32, fin2)
                nc.gpsimd.collective_compute(
                    "AllGather", mybir.AluOpType.bypass, replica_groups=rg,
                    ins=[bounce3[:]], outs=[t3[:]])

                # ---- L3 aggregate -> d (relu) -> L4 produce
                def fin3(i, acc3):
                    e1 = workp.tile([P, F1], mybir.dt.float32, tag="epi1")
                    nc.scalar.activation(e1[:], acc3[:], AF.Copy,
                                         scale=dinv_t[:, i : i + 1])
                    e2 = workp.tile([P, F1], mybir.dt.float32, tag="epi2")
                    nc.vector.tensor_tensor(
                        out=e2[:], in0=e1[:], in1=bias_t[:, OB1D : OB1D + F1],
                        op=mybir.AluOpType.add)
                    d = workp.tile([P, F1], mybir.dt.float32, tag="epi3")
                    nc.scalar.activation(d[:], e2[:], AF.Relu)
                    produce(i, d, F1, w2d_t, FO, bounce4, mybir.dt.float16)

                agg_blocks(t3, F1, mybir.dt.float16, fin3)
                nc.gpsimd.collective_compute(
                    "AllGather", mybir.AluOpType.bypass, replica_groups=rg,
                    ins=[bounce4[:]], outs=[t4[:]])

                # ---- L4 aggregate -> output
                def fin4(i, acc4):
                    o1 = workp.tile([P, FO], mybir.dt.float32, tag="o1")
                    nc.scalar.activation(o1[:], acc4[:], AF.Copy,
                                         scale=dinv_t[:, i : i + 1])
                    o2 = workp.tile([P, FO], mybir.dt.float32, tag="o2")
                    nc.vector.tensor_tensor(
                        out=o2[:], in0=o1[:], in1=bias_t[:, OB2D : OB2D + FO],
                        op=mybir.AluOpType.add)
                    nc.sync.dma_start(out=out_t[i * P : (i + 1) * P, :], in_=o2[:])

                agg_blocks(t4, FO, mybir.dt.float16, fin4)
    return nc


# ------------------------------------------------------------------ kernel --

_CACHE = {}


def kernel(x, edge_index, W1e, b1e, W2e, b2e, Wm, bm, ln_w, ln_b,
           W1d, b1d, W2d, b2d):
    x = np.asarray(x, dtype=np.float32)
    edge_index = np.asarray(edge_index)
    plan = build_plan(edge_index)
    old_of_new = plan["old_of_new"]
    real = old_of_new >= 0

    xg = np.zeros((NPAD, F1), np.float32)
    xg[real] = x[old_of_new[real]]
    bias_pack = np.zeros((P, F1 + F2 + FZ + F1 + FO + 2 * FZ), np.float32)
    o = 0
    for vec in (b1e, b2e, bm, b1d, b2d, ln_w, ln_b):
        v = np.asarray(vec, np.float32).ravel()
        bias_pack[:, o : o + v.size] = v[None, :]
        o += v.size

    in_maps = []
    for c in range(NCORES):
        in_maps.append({
            "x_shard": xg[c * SHARD : (c + 1) * SHARD],
            "idx": plan["core_idx"][c],
            "dinv_cols": plan["dinv_cols"][c],
            "W1e": np.asarray(W1e, np.float32),
            "W2e": np.asarray(W2e, np.float32),
            "Wm": np.asarray(Wm, np.float32),
            "W1d": np.asarray(W1d, np.float32),
            "W2d": np.asarray(W2d, np.float32),
            "biases": bias_pack,
            "ident": np.eye(P, dtype=np.float32),
        })

    key = (tuple(plan["waves"].ravel().tolist()), plan["cols16"])
    if key not in _CACHE:
        nc = build_program(plan["waves"], plan["cols16"])
        _CACHE[key] = SpmdRunner(nc)
    runner = _CACHE[key]
    runner.stage(in_maps)
    res = runner.results(runner.run())

    out_new = np.concatenate([res[c]["out"] for c in range(NCORES)], axis=0)
    out = np.zeros((N, FO), np.float32)
    out[old_of_new[real]] = out_new[real]
    return out
